# revision 17
# baseline (speedup 1.0000x reference)
"""MixedDecoder (moe_routing) Trainium2 Bass kernel, v3.

Data-parallel over batch: B=1024 split as 128 samples per core across 8
NeuronCores.  fp32 gate MLP + softmax; bf16 activation datapath; expert
weights stored in fp8-e3m4 (4-bit mantissa) and consumed directly by the
PE as the moving operand against bf16 stationary activations (mixed-dtype
matmul, validated bit-exact on HW).  Per-layer weight scale S_l (absmax
-> 14) is folded into the per-sample softmax-normalization reciprocal
that already scales each layer's PSUM result.

Per layer:  out = sum_e coeff[:,e] * (inp @ w[e]) + coeff @ b
PE-only accumulation: coeff scaling applied to K-transposed input tiles,
every (expert, k-tile) matmul plus the mixed-bias matmul accumulates into
one PSUM bank.  Ragged 64-row k-tiles of two adjacent experts are merged
into single [128,x] matmuls.

DMA: weights are host-packed and streamed on the sync queue in exact
consumption order (wz0, w0 pairs, wz1, w1 pairs, wz2, w2 pairs) starting
at the top of the program; small gate tensors ride the vector queue so
they land concurrently; bias on the scalar queue.  fp8 weights halve the
stream to ~6.3 MB/core, moving the kernel from DMA-bound to PE-bound.
"""

import numpy as np
import sys

sys.path.insert(0, "/opt/trn_rl_repo")

import concourse.bass as bass
import concourse.mybir as mybir
import concourse.tile as tile
from concourse.masks import make_identity

F32 = mybir.dt.float32
BF16 = mybir.dt.bfloat16
E3 = mybir.dt.float8e3
AF = mybir.ActivationFunctionType
ALU = mybir.AluOpType

B, LAT, FCON = 1024, 64, 256
IN_SZ = LAT + FCON              # 320
HID, E, GATE_H = 512, 8, 64
INTER = LAT + HID               # 576
OUT_SZ = 512
NCORES = 8
BL = B // NCORES                # 128

# which layers keep expert weights in fp8-e3m4 (False -> bf16)
LAYER_E3 = (True, True, False)
E3_TARGET = 14.0                # absmax after scaling (e3m4 max = 15.5)

LAST_EXEC_NS = None
LAST_RESULTS = None


def _split_multi_waits(bir_str):
    """Walrus accepts at most one sync wait per instruction; hoist extra
    on_wait entries onto standalone EventSemaphore instructions."""
    import json

    d = json.loads(bir_str)
    ctr = [0]

    def fix_list(lst):
        out = []
        for ins in lst:
            if isinstance(ins, dict) and "opcode" in ins and "sync_info" in ins:
                si = ins.get("sync_info") or {}
                ow = si.get("on_wait") or []
                if len(ow) > 1:
                    for w in ow[:-1]:
                        ctr[0] += 1
                        out.append({
                            "debug": ins.get("debug", 0),
                            "engine": ins["engine"],
                            "ins": [], "outs": [],
                            "name": f"splitwait_{ctr[0]}",
                            "opcode": "EventSemaphore",
                            "sync_info": {"on_update": [], "on_wait": [w]},
                        })
                    si["on_wait"] = [ow[-1]]
            out.append(ins)
        return out

    def walk(o):
        if isinstance(o, dict):
            for k, v in o.items():
                if (isinstance(v, list) and v and isinstance(v[0], dict)
                        and "opcode" in v[0]):
                    o[k] = fix_list(v)
                    for ins in o[k]:
                        walk(ins)
                else:
                    walk(v)
        elif isinstance(o, list):
            for v in o:
                walk(v)

    walk(d)
    return json.dumps(d).encode(), ctr[0]


def _install_wait_splitter():
    from concourse import bass2jax, bass_utils

    orig = bass_utils.compile_bir_kernel

    def wrapper(bir_str, *a, **k):
        if isinstance(bir_str, str):
            bir_str = bir_str.encode()
        new, n = _split_multi_waits(bir_str)
        return orig(new, *a, **k)

    bass2jax.compile_bir_kernel = wrapper
    return orig


def build_program(inv_scales):
    """inv_scales: per-layer 1/S_l to fold into the output reciprocal."""
    nc = bass.Bass()

    wdt = [E3 if e3 else BF16 for e3 in LAYER_E3]

    # bf16 transposed gate weights: gwa = [g1w t0 | g1w t1] on 128 partitions,
    # gwb = [g1w t2 | g2w | g3w] on 64 partitions, gbt = fp32 biases [64, 3].
    # smallpack [128, 784] bf16: cols 0:512 xb | 512:640 gwa | 640:776 gwb
    # (gwb valid on partitions 0:64) | 776:784 pad
    sp_d = nc.declare_dram_parameter("smallpack", [128, 784], BF16, isOutput=False)
    gbt_d = nc.declare_dram_parameter("gbt", [GATE_H, 3], F32, isOutput=False)
    w0_d = nc.declare_dram_parameter("w0f", [128, E * 1024], wdt[0], isOutput=False)
    w1_d = nc.declare_dram_parameter("w1f", [128, E * 2048], wdt[1], isOutput=False)
    w2_d = nc.declare_dram_parameter("w2f", [128, E * 2048], wdt[2], isOutput=False)
    wz0_d = nc.declare_dram_parameter("wz0", [128, 2048], wdt[0], isOutput=False)
    wz1_d = nc.declare_dram_parameter("wz1", [128, 2048], wdt[1], isOutput=False)
    wz2_d = nc.declare_dram_parameter("wz2", [128, 2048], wdt[2], isOutput=False)
    # ohbias [8, 3072] bf16: cols 0:1536 ohb | 1536:3072 biasb
    ohbias_d = nc.declare_dram_parameter("ohbias", [E, 3072], BF16, isOutput=False)
    out_d = nc.declare_dram_parameter("out", [BL, OUT_SZ], BF16, isOutput=True)

    with tile.TileContext(nc) as tc:
        with (
            tc.tile_pool(name="const", bufs=1) as cpool,
            tc.tile_pool(name="gate", bufs=1) as gpool,
            tc.tile_pool(name="acts", bufs=1) as apool,
            tc.tile_pool(name="elu", bufs=2) as epool,
            tc.tile_pool(name="wts", bufs=1) as wpool,
            tc.tile_pool(name="scaled", bufs=10) as spool,
            tc.tile_pool(name="zscaled", bufs=12) as zpool,
            tc.tile_pool(name="ps_main", bufs=2, space="PSUM") as ps_main,
            tc.tile_pool(name="ps_aux", bufs=2, space="PSUM") as ps_aux,
            tc.tile_pool(name="ps_tr", bufs=1, space="PSUM") as ps_tr_pool,
            tc.tile_pool(name="ps_bc", bufs=2, space="PSUM") as ps_bc,
            tc.tile_pool(name="ps_warm", bufs=1, space="PSUM") as ps_warm_pool,
        ):
            # ---- sync queue: gate-critical packed smalls, then the weight
            # stream in strict consumption order, all issued from the top so
            # DMA engines saturate as soon as the preamble ends.
            spt = cpool.tile([128, 784], BF16, tag="smallpack")
            nc.sync.dma_start(spt[:], sp_d[:])
            xb = spt[:, 0:512]
            gwa = spt[:, 512:640]
            gwb = spt[0:GATE_H, 640:776]
            gbt = gpool.tile([GATE_H, 3], F32, tag="gbt")
            nc.sync.dma_start(gbt[:], gbt_d[:])

            wz = []
            for li, wz_d in enumerate((wz0_d, wz1_d, wz2_d)):
                t = wpool.tile([128, 2048], wdt[li], tag=f"wz{li}")
                wz.append(t)
            w0t, w1t, w2t = [], [], []
            nc.sync.dma_start(wz[0][:], wz0_d[:])
            for p in range(4):  # expert pairs for w0: [128, 2*1024]
                t = wpool.tile([128, 2048], wdt[0], tag=f"w0p{p}")
                nc.sync.dma_start(t[:], w0_d[:, p * 2048 : (p + 1) * 2048])
                w0t.append(t)
            nc.sync.dma_start(wz[1][:], wz1_d[:])
            for p in range(4):  # expert pairs for w1: [128, 2*2048]
                t = wpool.tile([128, 4096], wdt[1], tag=f"w1p{p}")
                nc.sync.dma_start(t[:], w1_d[:, p * 4096 : (p + 1) * 4096])
                w1t.append(t)
            nc.sync.dma_start(wz[2][:], wz2_d[:])
            for p in range(4):
                t = wpool.tile([128, 4096], wdt[2], tag=f"w2p{p}")
                nc.sync.dma_start(t[:], w2_d[:, p * 4096 : (p + 1) * 4096])
                w2t.append(t)

            def wtile(lst, e, ntile):
                # expert e's k-tile view inside its pair tile
                return lst[e // 2][:, (e % 2) * ntile * 512 : (e % 2 + 1) * ntile * 512]

            # ---- scalar-table preload FIRST on the scalar queue: the gate
            # elu needs the Exp table, and the load costs ~1.3us.  Feed the
            # dummy activation from a gpsimd memset so it can't wait on the
            # identity constants.
            dummy_in = cpool.tile([1, 1], F32, tag="dummy_in")
            nc.gpsimd.memset(dummy_in[:], 0.0)
            dummy = cpool.tile([1, 1], F32, tag="dummy")
            nc.scalar.activation(dummy[:], dummy_in[:], AF.Exp)

            # ---- ohb + bias pack on the scalar queue (issued after the
            # table preload; needed only once coeff is ready)
            ohbias = cpool.tile([E, 3072], BF16, tag="ohbias")
            nc.scalar.dma_start(ohbias[:], ohbias_d[:])
            ohb = ohbias[:, 0:1536]
            biasb = ohbias[:, 1536:3072]

            ident = cpool.tile([128, 128], F32)
            make_identity(nc, ident[:])
            identb = cpool.tile([128, 128], BF16, tag="identb")
            make_identity(nc, identb[:])

            # PE warm-up: harmless matmuls fill every PE idle window so the
            # HAM clock gate unthrottles (1.2->2.4 GHz) early and never
            # re-throttles.  Dedicated PSUM bank so fillers never interact
            # with real work.
            ps_warm = ps_warm_pool.tile([128, 512], F32, tag="warm")

            def warm(n):
                for _ in range(n):
                    nc.tensor.matmul(ps_warm[:, 0:128], identb[:], identb[:],
                                     start=True, stop=True)

            warm(8)

            # ---- gate MLP, transposed bf16: h^T [64, 128] per layer, biases
            # folded into the scalar-engine activations (per-partition).
            def elu_t(ps_ap, out_tile, bias_ap):
                eg = epool.tile([GATE_H, BL], F32, tag="ge")
                rg = epool.tile([GATE_H, BL], F32, tag="gr")
                sg = epool.tile([GATE_H, BL], F32, tag="gs")
                nc.scalar.activation(eg[:], ps_ap, AF.Exp, bias=bias_ap)
                nc.vector.tensor_scalar(out=rg[:], in0=ps_ap, scalar1=bias_ap,
                                        scalar2=0.0, op0=ALU.add, op1=ALU.max)
                nc.scalar.activation(sg[:], eg[:], AF.Relu, bias=1.0, scale=-1.0)
                nc.vector.tensor_tensor(out=out_tile, in0=rg[:], in1=sg[:],
                                        op=ALU.subtract)

            ps_g = ps_aux.tile([GATE_H, BL], F32, tag="g")
            nc.tensor.matmul(ps_g[:], gwa[:, 0:GATE_H], xb[:, 0:128],
                             start=True, stop=False)
            nc.tensor.matmul(ps_g[:], gwa[:, GATE_H : 2 * GATE_H],
                             xb[:, 128:256], start=False, stop=False)
            nc.tensor.matmul(ps_g[:], gwb[:, 0:GATE_H], xb[0:GATE_H, 256:384],
                             start=False, stop=True)
            h1t = gpool.tile([GATE_H, BL], BF16, tag="h1t")
            elu_t(ps_g[:], h1t[:], gbt[:, 0:1])

            # keep the PE's activity window busy while the gate elu runs
            warm(4)

            ps_g2 = ps_aux.tile([GATE_H, BL], F32, tag="g")
            nc.tensor.matmul(ps_g2[:], gwb[:, GATE_H : 2 * GATE_H], h1t[:],
                             start=True, stop=True)
            h2t = gpool.tile([GATE_H, BL], BF16, tag="h2t")
            elu_t(ps_g2[:], h2t[:], gbt[:, 1:2])

            warm(4)

            ps_g3 = ps_aux.tile([E, BL], F32, tag="g")
            nc.tensor.matmul(ps_g3[:], gwb[:, 2 * GATE_H : 2 * GATE_H + E],
                             h2t[:], start=True, stop=True)
            # UNNORMALIZED softmax numerators (no max-subtraction: |logits|
            # is small).  The 1/sum(exp) normalization AND the per-layer fp8
            # weight scale are folded into each mixed layer's output scale,
            # so the gate critical path ends right here at ct.
            warm(4)
            enumt = gpool.tile([E, BL], F32, tag="enumt")
            nc.scalar.activation(enumt[:], ps_g3[:], AF.Exp,
                                 bias=gbt[0:E, 2:3])
            ct4b = gpool.tile([E, 128], BF16, tag="ct4b")
            nc.vector.tensor_copy(ct4b[:], enumt[:])

            # broadcast tiles: bcs[g][p, (j,b)] = coeff[b, 4g+j]; bcs[2] = pairs
            # (pairs first: they feed the z-pair scalings and first matmuls)
            bcs = [None, None, None]
            for g in (2, 0, 1):
                ps_b = ps_bc.tile([128, 512], F32, tag="bc")
                for j in range(4):
                    nc.tensor.matmul(
                        ps_b[:, j * 128 : (j + 1) * 128],
                        ohb[:, (g * 4 + j) * 128 : (g * 4 + j + 1) * 128],
                        ct4b[:],
                        start=True, stop=True,
                    )
                sb = apool.tile([128, 512], BF16, tag=f"bc{g}")
                if g == 2:
                    nc.vector.tensor_copy(sb[:], ps_b[:])
                else:
                    nc.scalar.activation(sb[:], ps_b[:], AF.Copy)
                bcs[g] = sb
            # fill the PE hole while DVE/gpsimd produce the scaled tiles
            warm(10)

            def bc_e(e):
                return bcs[e // 4][:, (e % 4) * 128 : (e % 4 + 1) * 128]

            def bc_pair(i):
                return bcs[2][:, i * 128 : (i + 1) * 128]

            # pre-scale all merged z-pair tiles (DVE, don't depend on h)
            zscaled = {}
            for li, src0 in [(0, 256), (1, 384), (2, 384)]:
                for i in range(4):
                    az = zpool.tile([128, 128], BF16, tag="az")
                    nc.gpsimd.tensor_tensor(
                        out=az[:], in0=xb[:, src0 : src0 + 128],
                        in1=bc_pair(i), op=ALU.mult,
                    )
                    zscaled[(li, i)] = az

            # softmax denominator -> rec [128,1]; off the critical path.
            # recs[l] = (1/sum(exp)) * (1/S_l) also undoes the fp8 scale.
            ps_en = ps_aux.tile([128, E], F32, tag="g")
            nc.tensor.transpose(ps_en[:, 0:E], enumt[:], ident[0:E, 0:E])
            esum = gpool.tile([128, 1], F32)
            nc.vector.tensor_reduce(esum[:], ps_en[:, 0:E],
                                    axis=mybir.AxisListType.X, op=ALU.add)
            rec = gpool.tile([128, 1], F32)
            nc.vector.reciprocal(rec[:], esum[:])
            recs = []
            for li in range(3):
                if inv_scales[li] == 1.0:
                    recs.append(rec)
                else:
                    rl = gpool.tile([128, 1], F32, tag=f"rec{li}")
                    nc.vector.tensor_scalar_mul(rl[:], rec[:],
                                                float(inv_scales[li]))
                    recs.append(rl)

            # ---- 3 mixed-expert layers.  Per layer: bias + z-pair matmuls
            # first (they only need ct), then the previous layer's h
            # transposes (PE filler ordering keeps the PE busy across the
            # elu chain), then the full k-tile matmuls.
            layer_cfg = [
                (w0t, 2, HID, True),
                (w1t, 4, HID, True),
                (w2t, 4, OUT_SZ, False),
            ]
            hc = None  # previous layer's h chunks (4 x [128,128] bf16)
            for li, (wt, ntile, NOUT, has_act) in enumerate(layer_cfg):
                ps_o = ps_main.tile([128, NOUT], F32)
                nc.tensor.matmul(
                    ps_o[:], ct4b[:],
                    biasb[:, li * 512 : (li + 1) * 512],
                    start=True, stop=False,
                )
                for i in range(4):
                    nc.tensor.matmul(
                        ps_o[:], zscaled[(li, i)][:],
                        wz[li][:, i * 512 : (i + 1) * 512],
                        start=False, stop=False,
                    )
                # fill the elu-chain latency at the layer boundary so the
                # HAM clock gate never sees an idle window
                if li > 0:
                    warm(8)
                # transpose previous layer's h chunks -> hT (after the
                # bias/z filler so the PE has work while elu completes);
                # expert 0/1's scaled tiles are chunked so their matmuls can
                # start as soon as each hT chunk lands.
                if li > 0:
                    ps_tr = ps_tr_pool.tile([128, 512], BF16, tag="ps_tr_h")
                    hT = apool.tile([128, 512], BF16, tag=f"hT{li}")
                    a0c, a1c = [], []
                    for t in range(4):
                        sl = slice(t * 128, (t + 1) * 128)
                        nc.tensor.transpose(ps_tr[:, sl], hc[t][:], identb[:])
                        nc.vector.tensor_copy(hT[:, sl], ps_tr[:, sl])
                        for ci, lst in ((0, a0c), (1, a1c)):
                            ac = spool.tile([128, 128], BF16, tag="a0c")
                            nc.vector.tensor_tensor(
                                out=ac[:], in0=hT[:, sl], in1=bc_e(ci),
                                op=ALU.mult,
                            )
                            lst.append(ac)
                src = xb[:, 0:256] if li == 0 else hT[:]
                for e in range(E):
                    if li > 0 and e < 2:
                        for t in range(ntile):
                            nc.tensor.matmul(
                                ps_o[:], (a0c if e == 0 else a1c)[t][:],
                                wtile(wt, e, ntile)[:, t * 512 : (t + 1) * 512],
                                start=False, stop=False,
                            )
                        continue
                    a = spool.tile([128, 512], BF16, tag="a")
                    nc.vector.tensor_tensor(
                        out=a[:, 0 : ntile * 128].rearrange(
                            "p (t b) -> p t b", t=ntile),
                        in0=src.rearrange("p (t b) -> p t b", t=ntile),
                        in1=bc_e(e).unsqueeze(1).broadcast_to([128, ntile, 128]),
                        op=ALU.mult,
                    )
                    for t in range(ntile):
                        nc.tensor.matmul(
                            ps_o[:], a[:, t * 128 : (t + 1) * 128],
                            wtile(wt, e, ntile)[:, t * 512 : (t + 1) * 512],
                            start=False, stop=(e == E - 1 and t == ntile - 1),
                        )

                if has_act:
                    # elu with the softmax normalization + fp8 weight scale
                    # folded in as a per-partition scale; full-width scalar
                    # activations (scalar ops have ~400ns fixed cost),
                    # chunked DVE subtracts so each transpose waits only on
                    # its chunk
                    e_ = epool.tile([128, NOUT], BF16, tag="elu_e")
                    r_ = epool.tile([128, NOUT], BF16, tag="elu_r")
                    s_ = epool.tile([128, NOUT], BF16, tag="elu_s")
                    nc.scalar.activation(e_[:], ps_o[:], AF.Exp,
                                         scale=recs[li][:])
                    nc.vector.tensor_scalar(out=r_[:], in0=ps_o[:],
                                            scalar1=recs[li][:], scalar2=0.0,
                                            op0=ALU.mult, op1=ALU.max)
                    nc.scalar.activation(s_[:], e_[:], AF.Relu,
                                         bias=1.0, scale=-1.0)
                    hc = []
                    for t in range(4):
                        sl = slice(t * 128, (t + 1) * 128)
                        ht = apool.tile([128, 128], BF16, tag=f"h{li}_{t}")
                        nc.vector.tensor_tensor(out=ht[:], in0=r_[:, sl],
                                                in1=s_[:, sl], op=ALU.subtract)
                        hc.append(ht)
                else:
                    res = apool.tile([128, NOUT], BF16, tag="res")
                    nc.vector.tensor_scalar_mul(res[:, 0:256], ps_o[:, 0:256],
                                                recs[li][:])
                    nc.sync.dma_start(out_d[:, 0:256], res[:, 0:256])
                    nc.scalar.activation(res[:, 256:512], ps_o[:, 256:512],
                                         AF.Copy, scale=recs[li][:])
                    nc.sync.dma_start(out_d[:, 256:512], res[:, 256:512])

    return nc


def prepare_in_maps(z, c, w0, b0, w1, b1, w2, b2,
                    gw1, gb1, gw2, gb2, gw3, gb3):
    import ml_dtypes
    bf = ml_dtypes.bfloat16
    e3np = ml_dtypes.float8_e3m4
    f = np.float32

    x = np.concatenate([z, c], axis=1).astype(f)                  # [B, 320]
    gwa = np.concatenate([gw1[0:128, :], gw1[128:256, :]], axis=1)  # [128,128]
    gwb = np.concatenate([gw1[256:320, :], gw2, gw3], axis=1)       # [64,136]
    gwb_pad = np.zeros((128, 136), f)
    gwb_pad[0:GATE_H] = gwb
    gbt = np.zeros((GATE_H, 3), f)
    gbt[:, 0] = gb1
    gbt[:, 1] = gb2
    gbt[0:E, 2] = gb3

    ws = [np.asarray(w, f) for w in (w0, w1, w2)]
    scales = []
    for li, w in enumerate(ws):
        if LAYER_E3[li]:
            s = E3_TARGET / float(np.abs(w).max())
        else:
            s = 1.0
        scales.append(s)
    wdtnp = [e3np if e3 else bf for e3 in LAYER_E3]

    def pack_full(li, r0, ntiles):
        # [128, E*ntiles*512]: col e*ntiles*512 + t*512 + o = w[e, r0+t*128+p, o]
        w = ws[li] * scales[li]
        return np.ascontiguousarray(
            w[:, r0 : r0 + ntiles * 128, :]
            .reshape(E, ntiles, 128, 512)
            .transpose(2, 0, 1, 3)
            .reshape(128, E * ntiles * 512)
            .astype(wdtnp[li])
        )

    def pack_z(li, r0):
        # [128, 4*512]: block i: p<64 -> w[2i, r0+p, :]; p>=64 -> w[2i+1, ...]
        w = ws[li] * scales[li]
        blocks = []
        for i in range(4):
            blocks.append(
                np.concatenate([w[2 * i, r0 : r0 + 64, :],
                                w[2 * i + 1, r0 : r0 + 64, :]], axis=0)
            )
        return np.ascontiguousarray(
            np.concatenate(blocks, axis=1).astype(wdtnp[li]))  # [128, 2048]

    w0f = pack_full(0, 0, 2)
    w1f = pack_full(1, 64, 4)
    w2f = pack_full(2, 64, 4)
    wz0 = pack_z(0, 256)
    wz1 = pack_z(1, 0)
    wz2 = pack_z(2, 0)
    biasb = np.concatenate(
        [np.asarray(b, f) * s for b, s in zip((b0, b1, b2), scales)],
        axis=1)                                                   # [8, 1536]

    ohb = np.zeros((E, 12 * 128), np.float32)
    for e in range(E):
        ohb[e, e * 128 : (e + 1) * 128] = 1.0
    for i in range(4):
        ohb[2 * i, (8 + i) * 128 : (8 + i) * 128 + 64] = 1.0
        ohb[2 * i + 1, (8 + i) * 128 + 64 : (9 + i) * 128] = 1.0
    ohbias = np.concatenate([ohb, biasb], axis=1).astype(bf)      # [8, 3072]

    shared = {
        "gbt": gbt,
        "w0f": w0f, "w1f": w1f, "w2f": w2f,
        "wz0": wz0, "wz1": wz1, "wz2": wz2,
        "ohbias": ohbias,
    }
    in_maps = []
    for i in range(NCORES):
        xT = np.ascontiguousarray(x.T[:, i * BL : (i + 1) * BL])  # [320, BL]
        xbb = np.concatenate([
            xT[0:128, :],
            xT[128:256, :],
            np.concatenate([xT[256:320, :], xT[256:320, :]], axis=0),
            np.concatenate([xT[0:64, :], xT[0:64, :]], axis=0),
        ], axis=1)                                                 # [128, 512]
        spk = np.concatenate(
            [xbb, gwa, gwb_pad, np.zeros((128, 8), f)], axis=1
        ).astype(bf)                                               # [128, 784]
        m = dict(shared)
        m["smallpack"] = spk
        in_maps.append(m)
    return in_maps, [1.0 / s for s in scales]


def kernel(z, c, w0, b0, w1, b1, w2, b2, gw1, gb1, gw2, gb2, gw3, gb3):
    global LAST_EXEC_NS, LAST_RESULTS
    from concourse.bass_utils import run_bass_kernel_spmd

    _install_wait_splitter()
    in_maps, inv_scales = prepare_in_maps(z, c, w0, b0, w1, b1, w2, b2,
                                          gw1, gb1, gw2, gb2, gw3, gb3)
    nc = build_program(inv_scales)
    r = run_bass_kernel_spmd(nc, in_maps, list(range(NCORES)))
    LAST_EXEC_NS = r.exec_time_ns
    LAST_RESULTS = r
    return np.concatenate(
        [np.asarray(r.results[i]["out"], np.float32) for i in range(NCORES)],
        axis=0)


# revision 22
# speedup vs baseline: 1.1789x; 1.1789x over previous
"""MixedDecoder (moe_routing) Trainium2 Bass kernel, v3.

Data-parallel over batch: B=1024 split as 128 samples per core across 8
NeuronCores.  fp32 gate MLP + softmax; bf16 activation datapath; expert
weights stored in fp8-e3m4 (4-bit mantissa) and consumed directly by the
PE as the moving operand against bf16 stationary activations (mixed-dtype
matmul, validated bit-exact on HW).  Per-layer weight scale S_l (absmax
-> 14) is folded into the per-sample softmax-normalization reciprocal
that already scales each layer's PSUM result.

Per layer:  out = sum_e coeff[:,e] * (inp @ w[e]) + coeff @ b
PE-only accumulation: coeff scaling applied to K-transposed input tiles,
every (expert, k-tile) matmul plus the mixed-bias matmul accumulates into
one PSUM bank.  Ragged 64-row k-tiles of two adjacent experts are merged
into single [128,x] matmuls.

DMA: weights are host-packed and streamed on the sync queue in exact
consumption order (wz0, w0 pairs, wz1, w1 pairs, wz2, w2 pairs) starting
at the top of the program; small gate tensors ride the vector queue so
they land concurrently; bias on the scalar queue.  fp8 weights halve the
stream to ~6.3 MB/core, moving the kernel from DMA-bound to PE-bound.
"""

import numpy as np
import sys

sys.path.insert(0, "/opt/trn_rl_repo")

import concourse.bass as bass
import concourse.mybir as mybir
import concourse.tile as tile
from concourse.masks import make_identity

F32 = mybir.dt.float32
BF16 = mybir.dt.bfloat16
E3 = mybir.dt.float8e3
AF = mybir.ActivationFunctionType
ALU = mybir.AluOpType

B, LAT, FCON = 1024, 64, 256
IN_SZ = LAT + FCON              # 320
HID, E, GATE_H = 512, 8, 64
INTER = LAT + HID               # 576
OUT_SZ = 512
NCORES = 8
BL = B // NCORES                # 128

# which layers keep expert weights in fp8-e3m4 (False -> bf16)
LAYER_E3 = (True, True, False)
E3_TARGET = 14.0                # absmax after scaling (e3m4 max = 15.5)

LAST_EXEC_NS = None
LAST_RESULTS = None


def _split_multi_waits(bir_str):
    """Walrus accepts at most one sync wait per instruction; hoist extra
    on_wait entries onto standalone EventSemaphore instructions."""
    import json

    d = json.loads(bir_str)
    ctr = [0]

    def fix_list(lst):
        out = []
        for ins in lst:
            if isinstance(ins, dict) and "opcode" in ins and "sync_info" in ins:
                si = ins.get("sync_info") or {}
                ow = si.get("on_wait") or []
                if len(ow) > 1:
                    for w in ow[:-1]:
                        ctr[0] += 1
                        out.append({
                            "debug": ins.get("debug", 0),
                            "engine": ins["engine"],
                            "ins": [], "outs": [],
                            "name": f"splitwait_{ctr[0]}",
                            "opcode": "EventSemaphore",
                            "sync_info": {"on_update": [], "on_wait": [w]},
                        })
                    si["on_wait"] = [ow[-1]]
            out.append(ins)
        return out

    def walk(o):
        if isinstance(o, dict):
            for k, v in o.items():
                if (isinstance(v, list) and v and isinstance(v[0], dict)
                        and "opcode" in v[0]):
                    o[k] = fix_list(v)
                    for ins in o[k]:
                        walk(ins)
                else:
                    walk(v)
        elif isinstance(o, list):
            for v in o:
                walk(v)

    walk(d)
    return json.dumps(d).encode(), ctr[0]


def _install_wait_splitter():
    from concourse import bass2jax, bass_utils

    orig = bass_utils.compile_bir_kernel

    def wrapper(bir_str, *a, **k):
        if isinstance(bir_str, str):
            bir_str = bir_str.encode()
        new, n = _split_multi_waits(bir_str)
        return orig(new, *a, **k)

    bass2jax.compile_bir_kernel = wrapper
    return orig


def build_program(inv_scales):
    """inv_scales: per-layer 1/S_l to fold into the output reciprocal."""
    nc = bass.Bass()

    wdt = [E3 if e3 else BF16 for e3 in LAYER_E3]

    # bf16 transposed gate weights: gwa = [g1w t0 | g1w t1] on 128 partitions,
    # gwb = [g1w t2 | g2w | g3w] on 64 partitions, gbt = fp32 biases [64, 3].
    # smallpack [128, 912] bf16: cols 0:512 xb | 512:640 gwa | 640:776 gwb
    # (gwb valid on partitions 0:64) | 776:904 identity | 904:912 zero pad
    sp_d = nc.declare_dram_parameter("smallpack", [128, 912], BF16, isOutput=False)
    gbt_d = nc.declare_dram_parameter("gbt", [GATE_H, 3], F32, isOutput=False)
    w0_d = nc.declare_dram_parameter("w0f", [128, E * 1024], wdt[0], isOutput=False)
    w1_d = nc.declare_dram_parameter("w1f", [128, E * 2048], wdt[1], isOutput=False)
    w2_d = nc.declare_dram_parameter("w2f", [128, E * 2048], wdt[2], isOutput=False)
    wz0_d = nc.declare_dram_parameter("wz0", [128, 2048], wdt[0], isOutput=False)
    wz1_d = nc.declare_dram_parameter("wz1", [128, 2048], wdt[1], isOutput=False)
    wz2_d = nc.declare_dram_parameter("wz2", [128, 2048], wdt[2], isOutput=False)
    # ohbias [8, 3072] bf16: cols 0:1536 ohb | 1536:3072 biasb
    ohbias_d = nc.declare_dram_parameter("ohbias", [E, 3072], BF16, isOutput=False)
    out_d = nc.declare_dram_parameter("out", [BL, OUT_SZ], BF16, isOutput=True)

    with tile.TileContext(nc) as tc:
        with (
            tc.tile_pool(name="const", bufs=1) as cpool,
            tc.tile_pool(name="gate", bufs=1) as gpool,
            tc.tile_pool(name="acts", bufs=1) as apool,
            tc.tile_pool(name="elu", bufs=2) as epool,
            tc.tile_pool(name="wts", bufs=1) as wpool,
            tc.tile_pool(name="scaled", bufs=10) as spool,
            tc.tile_pool(name="zscaled", bufs=12) as zpool,
            tc.tile_pool(name="ps_main", bufs=2, space="PSUM") as ps_main,
            tc.tile_pool(name="ps_aux", bufs=2, space="PSUM") as ps_aux,
            tc.tile_pool(name="ps_tr", bufs=1, space="PSUM") as ps_tr_pool,
            tc.tile_pool(name="ps_bc", bufs=2, space="PSUM") as ps_bc,
            tc.tile_pool(name="ps_warm", bufs=1, space="PSUM") as ps_warm_pool,
        ):
            # ---- sync queue: gate-critical packed smalls, then wz/w1/w2 in
            # consumption order.  w0 rides the gpsimd queue CONCURRENTLY so
            # layer 0's weights land early despite the ~600ns/issue pacing.
            spt = cpool.tile([128, 912], BF16, tag="smallpack")
            nc.sync.dma_start(spt[:], sp_d[:])
            xb = spt[:, 0:512]
            gwa = spt[:, 512:640]
            gwb = spt[0:GATE_H, 640:776]
            identb = spt[:, 776:904]
            gbt = gpool.tile([GATE_H, 3], F32, tag="gbt")
            nc.sync.dma_start(gbt[:], gbt_d[:])

            wz = []
            for li, wz_d in enumerate((wz0_d, wz1_d, wz2_d)):
                t = wpool.tile([128, 2048], wdt[li], tag=f"wz{li}")
                wz.append(t)
            w0t, w1t, w2t = [], [], []
            for p in range(4):  # expert pairs for w0 on gpsimd queue
                t = wpool.tile([128, 2048], wdt[0], tag=f"w0p{p}")
                nc.gpsimd.dma_start(t[:], w0_d[:, p * 2048 : (p + 1) * 2048])
                w0t.append(t)
            nc.sync.dma_start(wz[0][:], wz0_d[:])
            nc.sync.dma_start(wz[1][:], wz1_d[:])
            for p in range(4):  # expert pairs for w1: [128, 2*2048]
                t = wpool.tile([128, 4096], wdt[1], tag=f"w1p{p}")
                nc.sync.dma_start(t[:], w1_d[:, p * 4096 : (p + 1) * 4096])
                w1t.append(t)
            nc.sync.dma_start(wz[2][:], wz2_d[:])
            for p in range(4):
                t = wpool.tile([128, 4096], wdt[2], tag=f"w2p{p}")
                nc.sync.dma_start(t[:], w2_d[:, p * 4096 : (p + 1) * 4096])
                w2t.append(t)

            def wtile(lst, e, ntile):
                # expert e's k-tile view inside its pair tile
                return lst[e // 2][:, (e % 2) * ntile * 512 : (e % 2 + 1) * ntile * 512]

            # ---- scalar-table preload FIRST on the scalar queue: the gate
            # elu needs the Exp table, and the load costs ~1.3us.  The dummy
            # reads the smallpack zero pad (first data to land).
            dummy = cpool.tile([1, 1], F32, tag="dummy")
            nc.scalar.activation(dummy[:], spt[0:1, 904:905], AF.Exp)

            # ---- ohb + bias pack on the scalar queue (issued after the
            # table preload; needed only once coeff is ready)
            ohbias = cpool.tile([E, 3072], BF16, tag="ohbias")
            nc.scalar.dma_start(ohbias[:], ohbias_d[:])
            ohb = ohbias[:, 0:1536]
            biasb = ohbias[:, 1536:3072]

            # f32 identity (for the enumt transpose) from the bf16 one
            ident = cpool.tile([E, E], F32, tag="ident")
            nc.vector.tensor_copy(ident[:], identb[0:E, 0:E])

            # PE warm-up: harmless matmuls fill every PE idle window so the
            # HAM clock gate unthrottles (1.2->2.4 GHz) early and never
            # re-throttles.  Dedicated PSUM bank so fillers never interact
            # with real work.
            ps_warm = ps_warm_pool.tile([128, 512], F32, tag="warm")

            def warm(n):
                for _ in range(n):
                    nc.tensor.matmul(ps_warm[:, 0:128], identb[:], identb[:],
                                     start=True, stop=True)

            warm(8)

            # ---- gate MLP, transposed bf16: h^T [64, 128] per layer, biases
            # folded into the scalar-engine activations (per-partition).
            def elu_t(ps_ap, out_tile, bias_ap):
                eg = epool.tile([GATE_H, BL], F32, tag="ge")
                rg = epool.tile([GATE_H, BL], F32, tag="gr")
                sg = epool.tile([GATE_H, BL], F32, tag="gs")
                nc.scalar.activation(eg[:], ps_ap, AF.Exp, bias=bias_ap)
                nc.vector.tensor_scalar(out=rg[:], in0=ps_ap, scalar1=bias_ap,
                                        scalar2=0.0, op0=ALU.add, op1=ALU.max)
                nc.scalar.activation(sg[:], eg[:], AF.Relu, bias=1.0, scale=-1.0)
                nc.vector.tensor_tensor(out=out_tile, in0=rg[:], in1=sg[:],
                                        op=ALU.subtract)

            ps_g = ps_aux.tile([GATE_H, BL], F32, tag="g")
            nc.tensor.matmul(ps_g[:], gwa[:, 0:GATE_H], xb[:, 0:128],
                             start=True, stop=False)
            nc.tensor.matmul(ps_g[:], gwa[:, GATE_H : 2 * GATE_H],
                             xb[:, 128:256], start=False, stop=False)
            nc.tensor.matmul(ps_g[:], gwb[:, 0:GATE_H], xb[0:GATE_H, 256:384],
                             start=False, stop=True)
            h1t = gpool.tile([GATE_H, BL], BF16, tag="h1t")
            elu_t(ps_g[:], h1t[:], gbt[:, 0:1])

            # keep the PE's activity window busy while the gate elu runs
            warm(4)

            ps_g2 = ps_aux.tile([GATE_H, BL], F32, tag="g")
            nc.tensor.matmul(ps_g2[:], gwb[:, GATE_H : 2 * GATE_H], h1t[:],
                             start=True, stop=True)
            h2t = gpool.tile([GATE_H, BL], BF16, tag="h2t")
            elu_t(ps_g2[:], h2t[:], gbt[:, 1:2])

            warm(4)

            ps_g3 = ps_aux.tile([E, BL], F32, tag="g")
            nc.tensor.matmul(ps_g3[:], gwb[:, 2 * GATE_H : 2 * GATE_H + E],
                             h2t[:], start=True, stop=True)
            # UNNORMALIZED softmax numerators (no max-subtraction: |logits|
            # is small).  The 1/sum(exp) normalization AND the per-layer fp8
            # weight scale are folded into each mixed layer's output scale,
            # so the gate critical path ends right here at ct.
            warm(4)
            enumt = gpool.tile([E, BL], F32, tag="enumt")
            nc.scalar.activation(enumt[:], ps_g3[:], AF.Exp,
                                 bias=gbt[0:E, 2:3])
            ct4b = gpool.tile([E, 128], BF16, tag="ct4b")
            nc.vector.tensor_copy(ct4b[:], enumt[:])

            # broadcast tiles: bcs[g][p, (j,b)] = coeff[b, 4g+j]; bcs[2] = pairs
            # (pairs first: they feed the z-pair scalings and first matmuls)
            bcs = [None, None, None]
            for g in (2, 0, 1):
                ps_b = ps_bc.tile([128, 512], F32, tag="bc")
                for j in range(4):
                    nc.tensor.matmul(
                        ps_b[:, j * 128 : (j + 1) * 128],
                        ohb[:, (g * 4 + j) * 128 : (g * 4 + j + 1) * 128],
                        ct4b[:],
                        start=True, stop=True,
                    )
                sb = apool.tile([128, 512], BF16, tag=f"bc{g}")
                if g == 2:
                    nc.vector.tensor_copy(sb[:], ps_b[:])
                else:
                    nc.scalar.activation(sb[:], ps_b[:], AF.Copy)
                bcs[g] = sb
            # fill the PE hole while DVE/gpsimd produce the scaled tiles
            warm(10)

            def bc_e(e):
                return bcs[e // 4][:, (e % 4) * 128 : (e % 4 + 1) * 128]

            def bc_pair(i):
                return bcs[2][:, i * 128 : (i + 1) * 128]

            # pre-scale all merged z-pair tiles (DVE, don't depend on h)
            zscaled = {}
            for li, src0 in [(0, 256), (1, 384), (2, 384)]:
                for i in range(4):
                    az = zpool.tile([128, 128], BF16, tag="az")
                    nc.gpsimd.tensor_tensor(
                        out=az[:], in0=xb[:, src0 : src0 + 128],
                        in1=bc_pair(i), op=ALU.mult,
                    )
                    zscaled[(li, i)] = az

            # softmax denominator -> rec [128,1]; off the critical path.
            # recs[l] = (1/sum(exp)) * (1/S_l) also undoes the fp8 scale.
            ps_en = ps_aux.tile([128, E], F32, tag="g")
            nc.tensor.transpose(ps_en[:, 0:E], enumt[:], ident[:])
            esum = gpool.tile([128, 1], F32)
            nc.vector.tensor_reduce(esum[:], ps_en[:, 0:E],
                                    axis=mybir.AxisListType.X, op=ALU.add)
            rec = gpool.tile([128, 1], F32)
            nc.vector.reciprocal(rec[:], esum[:])
            recs = []
            for li in range(3):
                if inv_scales[li] == 1.0:
                    recs.append(rec)
                else:
                    rl = gpool.tile([128, 1], F32, tag=f"rec{li}")
                    nc.vector.tensor_scalar_mul(rl[:], rec[:],
                                                float(inv_scales[li]))
                    recs.append(rl)

            # ---- 3 mixed-expert layers.  Per layer: bias + z-pair matmuls
            # first (they only need ct), then the previous layer's h
            # transposes (PE filler ordering keeps the PE busy across the
            # elu chain), then the full k-tile matmuls.
            layer_cfg = [
                (w0t, 2, HID, True),
                (w1t, 4, HID, True),
                (w2t, 4, OUT_SZ, False),
            ]
            hc = None  # previous layer's h chunks (4 x [128,128] bf16)
            for li, (wt, ntile, NOUT, has_act) in enumerate(layer_cfg):
                ps_o = ps_main.tile([128, NOUT], F32)
                nc.tensor.matmul(
                    ps_o[:], ct4b[:],
                    biasb[:, li * 512 : (li + 1) * 512],
                    start=True, stop=False,
                )
                for i in range(4):
                    nc.tensor.matmul(
                        ps_o[:], zscaled[(li, i)][:],
                        wz[li][:, i * 512 : (i + 1) * 512],
                        start=False, stop=False,
                    )
                # fill the elu-chain latency at the layer boundary so the
                # HAM clock gate never sees an idle window
                if li > 0:
                    warm(8)
                # transpose previous layer's h chunks -> hT (after the
                # bias/z filler so the PE has work while elu completes);
                # expert 0/1's scaled tiles are chunked so their matmuls can
                # start as soon as each hT chunk lands.
                if li > 0:
                    ps_tr = ps_tr_pool.tile([128, 512], BF16, tag="ps_tr_h")
                    hT = apool.tile([128, 512], BF16, tag=f"hT{li}")
                    a0c, a1c = [], []
                    for t in range(4):
                        sl = slice(t * 128, (t + 1) * 128)
                        nc.tensor.transpose(ps_tr[:, sl], hc[t][:], identb[:])
                        nc.vector.tensor_copy(hT[:, sl], ps_tr[:, sl])
                        for ci, lst in ((0, a0c), (1, a1c)):
                            ac = spool.tile([128, 128], BF16, tag="a0c")
                            nc.vector.tensor_tensor(
                                out=ac[:], in0=hT[:, sl], in1=bc_e(ci),
                                op=ALU.mult,
                            )
                            lst.append(ac)
                src = xb[:, 0:256] if li == 0 else hT[:]
                for e in range(E):
                    if li > 0 and e < 2:
                        for t in range(ntile):
                            nc.tensor.matmul(
                                ps_o[:], (a0c if e == 0 else a1c)[t][:],
                                wtile(wt, e, ntile)[:, t * 512 : (t + 1) * 512],
                                start=False, stop=False,
                            )
                        continue
                    a = spool.tile([128, 512], BF16, tag="a")
                    nc.vector.tensor_tensor(
                        out=a[:, 0 : ntile * 128].rearrange(
                            "p (t b) -> p t b", t=ntile),
                        in0=src.rearrange("p (t b) -> p t b", t=ntile),
                        in1=bc_e(e).unsqueeze(1).broadcast_to([128, ntile, 128]),
                        op=ALU.mult,
                    )
                    for t in range(ntile):
                        nc.tensor.matmul(
                            ps_o[:], a[:, t * 128 : (t + 1) * 128],
                            wtile(wt, e, ntile)[:, t * 512 : (t + 1) * 512],
                            start=False, stop=(e == E - 1 and t == ntile - 1),
                        )

                if has_act:
                    # elu with the softmax normalization + fp8 weight scale
                    # folded in as a per-partition scale; full-width scalar
                    # activations (scalar ops have ~400ns fixed cost),
                    # chunked DVE subtracts so each transpose waits only on
                    # its chunk
                    e_ = epool.tile([128, NOUT], BF16, tag="elu_e")
                    r_ = epool.tile([128, NOUT], BF16, tag="elu_r")
                    s_ = epool.tile([128, NOUT], BF16, tag="elu_s")
                    nc.scalar.activation(e_[:], ps_o[:], AF.Exp,
                                         scale=recs[li][:])
                    nc.vector.tensor_scalar(out=r_[:], in0=ps_o[:],
                                            scalar1=recs[li][:], scalar2=0.0,
                                            op0=ALU.mult, op1=ALU.max)
                    nc.scalar.activation(s_[:], e_[:], AF.Relu,
                                         bias=1.0, scale=-1.0)
                    hc = []
                    for t in range(4):
                        sl = slice(t * 128, (t + 1) * 128)
                        ht = apool.tile([128, 128], BF16, tag=f"h{li}_{t}")
                        nc.vector.tensor_tensor(out=ht[:], in0=r_[:, sl],
                                                in1=s_[:, sl], op=ALU.subtract)
                        hc.append(ht)
                else:
                    res = apool.tile([128, NOUT], BF16, tag="res")
                    nc.vector.tensor_scalar_mul(res[:, 0:256], ps_o[:, 0:256],
                                                recs[li][:])
                    nc.sync.dma_start(out_d[:, 0:256], res[:, 0:256])
                    nc.scalar.activation(res[:, 256:512], ps_o[:, 256:512],
                                         AF.Copy, scale=recs[li][:])
                    nc.sync.dma_start(out_d[:, 256:512], res[:, 256:512])

    return nc


def prepare_in_maps(z, c, w0, b0, w1, b1, w2, b2,
                    gw1, gb1, gw2, gb2, gw3, gb3):
    import ml_dtypes
    bf = ml_dtypes.bfloat16
    e3np = ml_dtypes.float8_e3m4
    f = np.float32

    x = np.concatenate([z, c], axis=1).astype(f)                  # [B, 320]
    gwa = np.concatenate([gw1[0:128, :], gw1[128:256, :]], axis=1)  # [128,128]
    gwb = np.concatenate([gw1[256:320, :], gw2, gw3], axis=1)       # [64,136]
    gwb_pad = np.zeros((128, 136), f)
    gwb_pad[0:GATE_H] = gwb
    gbt = np.zeros((GATE_H, 3), f)
    gbt[:, 0] = gb1
    gbt[:, 1] = gb2
    gbt[0:E, 2] = gb3

    ws = [np.asarray(w, f) for w in (w0, w1, w2)]
    scales = []
    for li, w in enumerate(ws):
        if LAYER_E3[li]:
            s = E3_TARGET / float(np.abs(w).max())
        else:
            s = 1.0
        scales.append(s)
    wdtnp = [e3np if e3 else bf for e3 in LAYER_E3]

    def pack_full(li, r0, ntiles):
        # [128, E*ntiles*512]: col e*ntiles*512 + t*512 + o = w[e, r0+t*128+p, o]
        w = ws[li] * scales[li]
        return np.ascontiguousarray(
            w[:, r0 : r0 + ntiles * 128, :]
            .reshape(E, ntiles, 128, 512)
            .transpose(2, 0, 1, 3)
            .reshape(128, E * ntiles * 512)
            .astype(wdtnp[li])
        )

    def pack_z(li, r0):
        # [128, 4*512]: block i: p<64 -> w[2i, r0+p, :]; p>=64 -> w[2i+1, ...]
        w = ws[li] * scales[li]
        blocks = []
        for i in range(4):
            blocks.append(
                np.concatenate([w[2 * i, r0 : r0 + 64, :],
                                w[2 * i + 1, r0 : r0 + 64, :]], axis=0)
            )
        return np.ascontiguousarray(
            np.concatenate(blocks, axis=1).astype(wdtnp[li]))  # [128, 2048]

    w0f = pack_full(0, 0, 2)
    w1f = pack_full(1, 64, 4)
    w2f = pack_full(2, 64, 4)
    wz0 = pack_z(0, 256)
    wz1 = pack_z(1, 0)
    wz2 = pack_z(2, 0)
    biasb = np.concatenate(
        [np.asarray(b, f) * s for b, s in zip((b0, b1, b2), scales)],
        axis=1)                                                   # [8, 1536]

    ohb = np.zeros((E, 12 * 128), np.float32)
    for e in range(E):
        ohb[e, e * 128 : (e + 1) * 128] = 1.0
    for i in range(4):
        ohb[2 * i, (8 + i) * 128 : (8 + i) * 128 + 64] = 1.0
        ohb[2 * i + 1, (8 + i) * 128 + 64 : (9 + i) * 128] = 1.0
    ohbias = np.concatenate([ohb, biasb], axis=1).astype(bf)      # [8, 3072]

    shared = {
        "gbt": gbt,
        "w0f": w0f, "w1f": w1f, "w2f": w2f,
        "wz0": wz0, "wz1": wz1, "wz2": wz2,
        "ohbias": ohbias,
    }
    in_maps = []
    for i in range(NCORES):
        xT = np.ascontiguousarray(x.T[:, i * BL : (i + 1) * BL])  # [320, BL]
        xbb = np.concatenate([
            xT[0:128, :],
            xT[128:256, :],
            np.concatenate([xT[256:320, :], xT[256:320, :]], axis=0),
            np.concatenate([xT[0:64, :], xT[0:64, :]], axis=0),
        ], axis=1)                                                 # [128, 512]
        spk = np.concatenate(
            [xbb, gwa, gwb_pad, np.eye(128, dtype=f),
             np.zeros((128, 8), f)], axis=1
        ).astype(bf)                                               # [128, 912]
        m = dict(shared)
        m["smallpack"] = spk
        in_maps.append(m)
    return in_maps, [1.0 / s for s in scales]


def kernel(z, c, w0, b0, w1, b1, w2, b2, gw1, gb1, gw2, gb2, gw3, gb3):
    global LAST_EXEC_NS, LAST_RESULTS
    from concourse.bass_utils import run_bass_kernel_spmd

    _install_wait_splitter()
    in_maps, inv_scales = prepare_in_maps(z, c, w0, b0, w1, b1, w2, b2,
                                          gw1, gb1, gw2, gb2, gw3, gb3)
    nc = build_program(inv_scales)
    r = run_bass_kernel_spmd(nc, in_maps, list(range(NCORES)))
    LAST_EXEC_NS = r.exec_time_ns
    LAST_RESULTS = r
    return np.concatenate(
        [np.asarray(r.results[i]["out"], np.float32) for i in range(NCORES)],
        axis=0)
